# revision 5
# baseline (speedup 1.0000x reference)
"""Trainium2 Bass kernel for KNN OOD scoring (nn_KNNModel).

Computation (matches reference):
  queries = embeddings [B=4, D=128, 32, 32] -> 4096 per-pixel queries
  d(q, bank_i) euclidean, k=5 nearest, score = mean distance,
  bilinear upsample 32x32 -> 512x512.

Sharding: query-parallel over 8 cores. Core c owns batch c//2 and a
16-row band (c%2) of the 32x32 grid plus 1 halo row (17x32 = 544
queries), so each core resolves its queries' global top-5 against the
full bank with no cross-core communication, and emits its own
[256, 512] slab of the final upsampled output.

Per-core device algorithm, per 2048-column bank chunk:
  PSUM tile v = -|b|^2 (one bf16 matmul: all-(-1) stationary x squared
  bank) then += 2q.b (bf16 matmul). v = -(d^2) + |q|^2 per element.
  DVE max8 pulls the chunk's 8 largest v per query into a candidate
  buffer; after all chunks, max8-of-candidates gives the global top-8
  >= top-5. ScalarE computes sqrt(q2 - v) with a fused accumulate to
  get sum of the 5 smallest distances. The 1/5 is folded into the
  bilinear weights; upsampling runs as two small fp32 matmuls.
"""

import os
import time

import numpy as np
import ml_dtypes

import concourse.bass as bass
from concourse import bacc
import concourse.mybir as mybir
import concourse.tile as tile
from concourse.bass_utils import run_bass_kernel_spmd

# ---- problem constants (hardcoded per contract) ----
B, D, H, W = 4, 128, 32, 32
N_BANK = 50000
K_NN = 5
OUT_H = OUT_W = 512

CHUNK = 2048
NCHUNKS = 25
NPAD = CHUNK * NCHUNKS          # 51200
HALO_ROWS = 17                  # 16-row band + 1 halo row
QPC = HALO_ROWS * W             # 544 queries per core
QTILES = 5
QPAD = QTILES * 128             # 640

LAST_EXEC_NS = None


def _resize_weight(out_size, in_size):
    """jax.image.resize(method='bilinear') triangle-kernel weights."""
    scale = out_size / in_size
    sample_f = (np.arange(out_size) + 0.5) / scale - 0.5
    x = np.abs(sample_f[:, None] - np.arange(in_size)[None, :])
    w = np.maximum(0.0, 1.0 - x)
    w = w / w.sum(axis=1, keepdims=True)
    return w.astype(np.float32)  # [out, in]


def build_kernel(nchunks=NCHUNKS, chunk=CHUNK):
    """Build the per-core SPMD Bass program. Returns compiled nc."""
    npad = nchunks * chunk
    nc = bacc.Bacc("TRN2", target_bir_lowering=False)
    f32 = mybir.dt.float32
    bf16 = mybir.dt.bfloat16

    qt2_d = nc.dram_tensor("qt2", [D, QPAD], bf16, kind="ExternalInput")
    bankT_d = nc.dram_tensor("bankT", [D, npad], bf16, kind="ExternalInput")
    rhT_d = nc.dram_tensor("rhT", [W, OUT_W], f32, kind="ExternalInput")
    rvT_d = nc.dram_tensor("rvT", [HALO_ROWS, 256], f32, kind="ExternalInput")
    out_d = nc.dram_tensor("out", [256, OUT_W], f32, kind="ExternalOutput")
    scratch_d = nc.dram_tensor("scratch", [QTILES, 128, 1], f32)

    with tile.TileContext(nc) as tc:
        with (
            tc.tile_pool(name="sb", bufs=3) as sb,
            tc.tile_pool(name="pers", bufs=1) as pers,
            tc.tile_pool(name="ps", bufs=2, space="PSUM") as ps,
        ):
            # constants
            negones = pers.tile([128, 128], bf16, tag="negones")
            nc.vector.memset(negones[:], -1.0)
            onescol = pers.tile([128, 1], bf16, tag="onescol")
            nc.vector.memset(onescol[:], 1.0)

            # queries (stationary side): 2*q^T, bf16
            qt2 = pers.tile([D, QPAD], bf16, tag="qt2")
            nc.sync.dma_start(out=qt2[:], in_=qt2_d.ap())
            # squared queries for |q|^2
            sqq = pers.tile([D, QPAD], bf16, tag="sqq")
            nc.scalar.activation(sqq[:], qt2[:], mybir.ActivationFunctionType.Square)

            # per-qtile |q|^2 = 0.25 * colsum((2q)^2): matmul with ones column
            q2t = []
            for t in range(QTILES):
                q2ps = ps.tile([128, 1], f32, tag="ps")
                nc.tensor.matmul(
                    out=q2ps[:],
                    lhsT=sqq[:, t * 128:(t + 1) * 128],
                    rhs=onescol[:],
                    start=True,
                    stop=True,
                )
                q2 = pers.tile([128, 1], f32, tag=f"q2_{t}")
                nc.scalar.activation(
                    q2[:], q2ps[:], mybir.ActivationFunctionType.Copy, scale=0.25
                )
                q2t.append(q2)

            # candidate buffers: per qtile, 8 values per chunk
            cand = [
                pers.tile([128, 8 * nchunks], f32, tag=f"cand_{t}",
                          name=f"cand_{t}")
                for t in range(QTILES)
            ]

            nq = chunk // 512  # matmul output must fit one PSUM bank (512 f32)
            for c in range(nchunks):
                bk = sb.tile([D, chunk], bf16, tag="bk")
                nc.sync.dma_start(out=bk[:], in_=bankT_d.ap()[:, c * chunk:(c + 1) * chunk])
                sq = sb.tile([D, chunk], bf16, tag="sq")
                nc.scalar.activation(sq[:], bk[:], mybir.ActivationFunctionType.Square)

                for t in range(QTILES):
                    pst = ps.tile([128, chunk], f32, tag="ps")
                    for h in range(nq):
                        sl = slice(h * 512, (h + 1) * 512)
                        # v = -|b|^2  (broadcast via all-(-1) stationary)
                        nc.tensor.matmul(
                            out=pst[:, sl],
                            lhsT=negones[:],
                            rhs=sq[:, sl],
                            start=True,
                            stop=False,
                        )
                        # v += 2 q . b
                        nc.tensor.matmul(
                            out=pst[:, sl],
                            lhsT=qt2[:, t * 128:(t + 1) * 128],
                            rhs=bk[:, sl],
                            start=False,
                            stop=True,
                        )
                    nc.vector.max(cand[t][:, c * 8:(c + 1) * 8], pst[:])

            # tail: global top8 -> 5 smallest distances -> summed
            for t in range(QTILES):
                top8 = sb.tile([128, 8], f32, tag="top8")
                nc.vector.max(top8[:], cand[t][:])
                d5 = sb.tile([128, K_NN], f32, tag="d5")
                ssum = sb.tile([128, 1], f32, tag="ssum")
                nc.scalar.activation(
                    d5[:],
                    top8[:, 0:K_NN],
                    mybir.ActivationFunctionType.Sqrt,
                    scale=-1.0,
                    bias=q2t[t][:],
                    accum_out=ssum[:],
                )
                nc.sync.dma_start(out=scratch_d.ap()[t], in_=ssum[:])

            # bilinear resize: out = Rv @ S @ Rh^T (1/5 folded into rhT)
            s_t = sb.tile([W, HALO_ROWS], f32, tag="s_t")
            src = scratch_d.ap().rearrange("t p one -> (t p one)")
            src = src.rearrange("(r c) -> c r", c=W)[:, :HALO_ROWS]
            nc.sync.dma_start(out=s_t[:], in_=src)

            rhT = pers.tile([W, OUT_W], f32, tag="rhT")
            nc.sync.dma_start(out=rhT[:], in_=rhT_d.ap())
            rvT = pers.tile([HALO_ROWS, 256], f32, tag="rvT")
            nc.sync.dma_start(out=rvT[:], in_=rvT_d.ap())

            aps = ps.tile([HALO_ROWS, OUT_W], f32, tag="ps")
            nc.tensor.matmul(out=aps[:], lhsT=s_t[:], rhs=rhT[:], start=True, stop=True)
            a_sb = sb.tile([HALO_ROWS, OUT_W], f32, tag="a_sb")
            nc.scalar.activation(a_sb[:], aps[:], mybir.ActivationFunctionType.Copy)

            for hh in range(2):
                ops = ps.tile([128, OUT_W], f32, tag="ps")
                nc.tensor.matmul(
                    out=ops[:],
                    lhsT=rvT[:, hh * 128:(hh + 1) * 128],
                    rhs=a_sb[:],
                    start=True,
                    stop=True,
                )
                o_sb = sb.tile([128, OUT_W], f32, tag="o_sb", name="o_sb")
                nc.scalar.activation(
                    o_sb[:], ops[:], mybir.ActivationFunctionType.Copy
                )
                nc.sync.dma_start(
                    out=out_d.ap()[hh * 128:(hh + 1) * 128, :], in_=o_sb[:]
                )

    nc.compile()
    return nc


def make_in_maps(embeddings, bank, nchunks=NCHUNKS, chunk=CHUNK):
    """Host-side shard prep: per-core input dict."""
    npad = nchunks * chunk
    n_items = min(N_BANK, npad)
    bankT = np.full([D, npad], 1000.0, dtype=ml_dtypes.bfloat16)
    bankT[:, :n_items] = bank[:n_items].T.astype(ml_dtypes.bfloat16)

    wh = _resize_weight(OUT_W, W)              # [512, 32]
    wv = _resize_weight(OUT_H, H)              # [512, 32]
    rhT = np.ascontiguousarray((wh * (1.0 / K_NN)).T)  # [32, 512]

    in_maps = []
    for c in range(8):
        b, band = c // 2, c % 2
        r0 = band * 15
        q = embeddings[b][:, r0:r0 + HALO_ROWS, :].reshape(D, QPC)
        qt2 = np.zeros([D, QPAD], dtype=ml_dtypes.bfloat16)
        qt2[:, :QPC] = (2.0 * q).astype(ml_dtypes.bfloat16)
        wv_band = wv[band * 256:(band + 1) * 256, r0:r0 + HALO_ROWS]  # [256, 17]
        rvT = np.ascontiguousarray(wv_band.T)  # [17, 256]
        in_maps.append({
            "qt2": qt2,
            "bankT": bankT,
            "rhT": rhT,
            "rvT": rvT,
        })
    return in_maps


_NC_CACHE = {}


def kernel(embeddings, bank, k, out_h, out_w):
    global LAST_EXEC_NS
    embeddings = np.asarray(embeddings, dtype=np.float32)
    bank = np.asarray(bank, dtype=np.float32)
    assert int(k) == K_NN and int(out_h) == OUT_H and int(out_w) == OUT_W
    assert embeddings.shape == (B, D, H, W) and bank.shape == (N_BANK, D)

    if "nc" not in _NC_CACHE:
        _NC_CACHE["nc"] = build_kernel()
    nc = _NC_CACHE["nc"]

    in_maps = make_in_maps(embeddings, bank)
    trace = bool(int(os.environ.get("KNN_TRACE", "0")))
    t0 = time.time()
    res = run_bass_kernel_spmd(nc, in_maps, list(range(8)), trace=trace)
    t1 = time.time()
    LAST_EXEC_NS = res.exec_time_ns if res.exec_time_ns else int((t1 - t0) * 1e9)

    full = np.zeros([B, 1, OUT_H, OUT_W], dtype=np.float32)
    for c in range(8):
        b, band = c // 2, c % 2
        full[b, 0, band * 256:(band + 1) * 256, :] = res.results[c]["out"]
    return full
